# revision 2
# baseline (speedup 1.0000x reference)
"""Trainium2 Bass kernel: BERT-style self-attention with per-task additive
embeddings (B=4, S=2048, H=1024, 16 heads x 64 dim).

Sharding (8 NeuronCores): core = (batch b, head-group hg), b = core//2,
hg = core%2. Each core: full S^2 attention for its batch and its 8 heads.

v2 design (vs 337us baseline):
  - Score matmuls (K=64) issued as row-tiled pairs (PE tiles T0/T8 via
    lhsT base partitions 0/64) -> the two matmuls run CONCURRENTLY on
    disjoint 64-row halves of the PE array (measured 116ns/mm vs 216).
  - ctx matmul M=65: a ones-column appended to V (vaug[:, h*65+64])
    makes PSUM row 64 accumulate sum_k P[k,q] = the softmax denominator,
    in fp32, for free -> the entire DVE dacc chain + den matmuls of the
    baseline are gone.
  - PE instruction stream is emitted in CHUNKS: runs of score pairs
    (64-row mode) alternating with runs of ctx/proj matmuls (128-row
    mode), since fine-grained mode interleave serializes the PE
    (measured: 219ns/mm fine vs 174 chunked-3).
  - A small fraction of exp tiles (kb in DVE_KBS) is computed on the
    DVE instead of ScalarE via a Schraudolph-style exp: in the bf16-bits
    domain, rne(s*128*log2e + 128*(127-C)) IS the bf16 bit pattern of
    ~exp(s) (rel rms ~1.8%). One tensor_scalar op writing through an
    int16 bitcast of the bf16 pt tile. Applied to 2/16 of tiles ->
    ctx rel err contribution ~0.6%, total comfortably < 2e-2 gate.
  - ScalarE's exp table set is loaded by a dummy ACTIVATE at t=0 so the
    ~2.7us table load overlaps the initial DMAs.

PSUM budget (8 banks): st 2x2 + ctx 2x1 + proj 2x1 = 8.
Host: fold biases/task-emb into weights' bias row equivalents (bq/bk
into post-matmul adds, bv added host-side after normalize), divide ctx
rows by den, transpose into [B,S,H].
"""

import numpy as np
import ml_dtypes
from collections import deque
from contextlib import ExitStack
import heapq

B, S, H = 4, 2048, 1024
NH, HD = 16, 64
P = 128
NKB = H // P          # 8 contraction blocks for projections
NTB = S // P          # 16 key blocks
NQC = S // 512        # 4 query chunks
NPAIR = 4             # head pairs per core
HPC = 8               # heads per core
OUTROWS = HPC * HD    # 512
JC = 512              # weight columns per core

CHUNK = 4             # score-pair slots per 128-mode run
LAG = 16              # ctx pops lag (slots) behind its exp emission
DVE_KBS = (6, 13)     # kb whose exp runs on DVE (Schraudolph)
EXP_A = 184.6649652   # 128*log2(e)
EXP_B = 16248.577     # 128*(127-0.058)

_CACHE = {}


def _build():
    import concourse.mybir as mybir
    import concourse.tile as tile
    from concourse import bacc

    f32 = mybir.dt.float32
    bf16 = mybir.dt.bfloat16
    i16 = mybir.dt.int16
    EXP = mybir.ActivationFunctionType.Exp
    MULT = mybir.AluOpType.mult
    ADD = mybir.AluOpType.add

    nc = bacc.Bacc("TRN2", target_bir_lowering=False, debug=False,
                   enable_asserts=True)
    hsT = nc.dram_tensor("hsT", [H, S], bf16, kind="ExternalInput").ap()
    wq = nc.dram_tensor("wq", [H, JC], bf16, kind="ExternalInput").ap()
    wk = nc.dram_tensor("wk", [H, JC], bf16, kind="ExternalInput").ap()
    wv = nc.dram_tensor("wv", [H, JC], bf16, kind="ExternalInput").ap()
    em = nc.dram_tensor("em", [P, NTB], f32, kind="ExternalInput").ap()
    bqk = nc.dram_tensor("bqk", [P, 8], f32, kind="ExternalInput").ap()
    out = nc.dram_tensor("out", [OUTROWS, S], f32, kind="ExternalOutput").ap()
    den = nc.dram_tensor("den", [HPC * NQC, 512], f32,
                         kind="ExternalOutput").ap()

    with tile.TileContext(nc) as tc:
        with ExitStack() as ctx:
            const = ctx.enter_context(tc.tile_pool(name="const", bufs=1))
            wpool = ctx.enter_context(tc.tile_pool(name="wpool", bufs=1))
            hpool = ctx.enter_context(tc.tile_pool(name="hpool", bufs=1))
            qkpool = ctx.enter_context(tc.tile_pool(name="qkpool", bufs=1))
            vpool = ctx.enter_context(tc.tile_pool(name="vpool", bufs=1))
            ptpool = ctx.enter_context(tc.tile_pool(name="ptpool", bufs=32))
            stgpool = ctx.enter_context(tc.tile_pool(name="stgpool", bufs=4))
            psst = ctx.enter_context(
                tc.tile_pool(name="psst", bufs=2, space="PSUM"))
            psctx = ctx.enter_context(
                tc.tile_pool(name="psctx", bufs=2, space="PSUM"))
            psproj = ctx.enter_context(
                tc.tile_pool(name="psproj", bufs=2, space="PSUM"))

            # consts + ACT exp-table warm (overlaps initial DMA)
            emask = const.tile([P, NTB], f32, tag="emask", name="emask")
            nc.sync.dma_start(emask[:], em)
            bqkcol = const.tile([P, 8], f32, tag="bqkcol", name="bqkcol")
            nc.sync.dma_start(bqkcol[:], bqk)
            warm = const.tile([P, 2], f32, tag="warm", name="warm")
            nc.vector.memset(warm[:], 0.0)
            nc.scalar.activation(warm[:, 1:2], warm[:, 0:1], EXP)

            # vaug tiles: [128, 8*65], ones at col h*65+64
            vaug = [vpool.tile([P, HPC * 65], bf16, tag=f"vaug{tb}",
                               name=f"vaug{tb}") for tb in range(NTB)]
            for tb in range(NTB):
                va65 = vaug[tb][:].rearrange("p (h c) -> p h c", c=65)
                nc.vector.memset(va65[:, :, 64:65], 1.0)

            # ---- DMA emission ordered by first use ----
            hst = [hpool.tile([P, S], bf16, tag=f"hst{kb}", name=f"hst{kb}")
                   for kb in range(NKB)]
            wt = {}
            for name, dram in (("k", wk), ("v", wv), ("q", wq)):
                wt[name] = [wpool.tile([P, JC], bf16, tag=f"w{name}{kb}",
                                       name=f"w{name}{kb}")
                            for kb in range(NKB)]
            # first: wk/wq m0 columns + hsT chunk 0 (feed prologue chains)
            for kb in range(NKB):
                nc.sync.dma_start(wt["k"][kb][:, 0:P], wk[kb * P:(kb + 1) * P, 0:P])
                nc.gpsimd.dma_start(wt["q"][kb][:, 0:P], wq[kb * P:(kb + 1) * P, 0:P])
                nc.scalar.dma_start(
                    hst[kb][:, 0:512], hsT[kb * P:(kb + 1) * P, 0:512])
            # wv (v chains start ~slot 8), then hst tci1-3 (needed by the
            # qc1-3 q-chains and v-chains from ~slot 12), then the
            # remaining wk/wq columns (pair m1 chains, ~slot 58)
            for kb in range(NKB):
                nc.sync.dma_start(wt["v"][kb][:], wv[kb * P:(kb + 1) * P, :])
            for tci in range(1, 4):
                for kb in range(NKB):
                    eng = (nc.scalar, nc.gpsimd)[(tci * NKB + kb) % 2]
                    eng.dma_start(
                        hst[kb][:, tci * 512:(tci + 1) * 512],
                        hsT[kb * P:(kb + 1) * P, tci * 512:(tci + 1) * 512])
            for kb in range(NKB):
                nc.sync.dma_start(wt["k"][kb][:, P:JC],
                                  wk[kb * P:(kb + 1) * P, P:JC])
                nc.gpsimd.dma_start(wt["q"][kb][:, P:JC],
                                    wq[kb * P:(kb + 1) * P, P:JC])

            # ---- projection chain emitters (split in 2 parts of 4 mms) ----
            vchain_ps = {}

            def v_chain(tb, part):
                if part == 0:
                    ps = psproj.tile([P, JC], f32, tag="psproj", name="psv")
                    vchain_ps[tb] = ps
                    for kb in range(4):
                        nc.tensor.matmul(ps[:],
                                         lhsT=hst[kb][:, tb * P:(tb + 1) * P],
                                         rhs=wt["v"][kb][:],
                                         start=(kb == 0), stop=False)
                else:
                    ps = vchain_ps.pop(tb)
                    for kb in range(4, NKB):
                        nc.tensor.matmul(ps[:],
                                         lhsT=hst[kb][:, tb * P:(tb + 1) * P],
                                         rhs=wt["v"][kb][:],
                                         start=False, stop=(kb == NKB - 1))
                    va = vaug[tb][:].rearrange(
                        "p (h c) -> p h c", c=65)[:, :, 0:64]
                    pv = ps[:].rearrange("p (h d) -> p h d", d=HD)
                    sc = emask[:, tb:tb + 1]
                    nc.vector.tensor_scalar_mul(va, pv, sc)

            qT = [qkpool.tile([P, S], bf16, tag=f"qT{m}", name=f"qT{m}")
                  for m in range(NPAIR)]
            kT = [qkpool.tile([P, S], bf16, tag=f"kT{m}", name=f"kT{m}")
                  for m in range(NPAIR)]

            qkchain_ps = {}

            def qk_chain(name, m, tci, part):
                dst = (qT if name == "q" else kT)[m]
                if part == 0:
                    ps = psproj.tile([P, 512], f32, tag="psproj", name="psqk")
                    qkchain_ps[(name, m, tci)] = ps
                    for kb in range(4):
                        nc.tensor.matmul(
                            ps[:],
                            lhsT=wt[name][kb][:, m * P:(m + 1) * P],
                            rhs=hst[kb][:, tci * 512:(tci + 1) * 512],
                            start=(kb == 0), stop=False)
                else:
                    ps = qkchain_ps.pop((name, m, tci))
                    for kb in range(4, NKB):
                        nc.tensor.matmul(
                            ps[:],
                            lhsT=wt[name][kb][:, m * P:(m + 1) * P],
                            rhs=hst[kb][:, tci * 512:(tci + 1) * 512],
                            start=False, stop=(kb == NKB - 1))
                    bc = bqkcol[:, (0 if name == "q" else 4) + m:
                                (1 if name == "q" else 5) + m]
                    nc.vector.tensor_scalar_add(
                        dst[:, tci * 512:(tci + 1) * 512], ps[:], bc)

            # ---- projection piece queue: (due_slot, seq, fn, args) ----
            projq = []
            seq = [0]

            def padd(due, fn, *args):
                heapq.heappush(projq, (due, seq[0], fn, args))
                seq[0] += 1

            # prologue chains emitted immediately: kT[0] tci0, qT[0] qc0
            qk_chain("k", 0, 0, 0)
            qk_chain("k", 0, 0, 1)
            qk_chain("q", 0, 0, 0)
            qk_chain("q", 0, 0, 1)

            for m in range(NPAIR):
                for tci in range(4):
                    if m == 0 and tci == 0:
                        continue
                    due = m * 64 + 4 * tci
                    padd(due - 6, qk_chain, "k", m, tci, 0)
                    padd(due - 3, qk_chain, "k", m, tci, 1)
                for qc2 in range(NQC):
                    if m == 0 and qc2 == 0:
                        continue
                    due = (m * 4 + qc2) * 16
                    padd(due - 6, qk_chain, "q", m, qc2, 0)
                    padd(due - 3, qk_chain, "q", m, qc2, 1)
            for tb in range(NTB):
                due = 16 + tb
                padd(due - 8, v_chain, tb, 0)
                padd(due - 4, v_chain, tb, 1)

            # ---- ctx emission machinery ----
            pend_ctx = deque()   # (slot_emitted, m, qc, kb, pt)
            ctx_tiles = {}

            def emit_ctx(m, qc, kb, pt):
                key = (m, qc)
                if key not in ctx_tiles:
                    c0 = psctx.tile([65, 512], f32, tag="ctx", name="c0")
                    c1 = psctx.tile([65, 512], f32, tag="ctx", name="c1")
                    ctx_tiles[key] = (c0, c1)
                c0, c1 = ctx_tiles[key]
                for h, ct in ((0, c0), (1, c1)):
                    head = 2 * m + h
                    nc.tensor.matmul(
                        ct[:],
                        lhsT=vaug[kb][:, head * 65:(head + 1) * 65],
                        rhs=pt[:, h * 512:(h + 1) * 512],
                        start=(kb == 0), stop=(kb == NTB - 1),
                        skip_group_check=True)
                if kb == NTB - 1:
                    for h, ct in ((0, c0), (1, c1)):
                        head = 2 * m + h
                        stg = stgpool.tile([65, 512], f32, tag="stg",
                                           name="stg")
                        nc.vector.tensor_copy(stg[:], ct[:])
                        nc.sync.dma_start(
                            out[head * HD:(head + 1) * HD,
                                qc * 512:(qc + 1) * 512], stg[0:64, :])
                        nc.gpsimd.dma_start(
                            den[head * NQC + qc:head * NQC + qc + 1, :],
                            stg[64:65, :])
                    del ctx_tiles[key]

            def run128(t, budget):
                used = 0
                while projq and projq[0][0] <= t + 6:
                    _, _, fn, args = heapq.heappop(projq)
                    fn(*args)
                    used += 4
                while pend_ctx and used < budget:
                    ts, m2, qc2, kb2, pt2 = pend_ctx[0]
                    if ts + LAG > t:
                        break
                    pend_ctx.popleft()
                    emit_ctx(m2, qc2, kb2, pt2)
                    used += 2
                while projq and used < budget - 3:
                    _, _, fn, args = heapq.heappop(projq)
                    fn(*args)
                    used += 4

            # ---- main loop ----
            for g in range(16):
                m, qc = divmod(g, 4)
                for kb in range(NTB):
                    t = g * 16 + kb
                    st = psst.tile([P, 1024], f32, tag="st", name="st")
                    nc.tensor.matmul(
                        st[:, 0:512],
                        lhsT=kT[m][0:64, kb * P:(kb + 1) * P],
                        rhs=qT[m][0:64, qc * 512:(qc + 1) * 512],
                        start=True, stop=True)
                    nc.tensor.matmul(
                        st[:, 512:1024],
                        lhsT=kT[m][64:128, kb * P:(kb + 1) * P],
                        rhs=qT[m][64:128, qc * 512:(qc + 1) * 512],
                        start=True, stop=True)
                    pt = ptpool.tile([P, 1024], bf16, tag="pt", name="pt")
                    if kb in DVE_KBS:
                        nc.vector.tensor_scalar(
                            pt[:].bitcast(i16), st[:], EXP_A, EXP_B,
                            MULT, ADD)
                    else:
                        nc.scalar.activation(pt[:], st[:], EXP)
                    pend_ctx.append((t, m, qc, kb, pt))
                    if kb % CHUNK == CHUNK - 1:
                        run128(t, 10)

            # tail: flush everything
            t = 256
            while projq or pend_ctx:
                run128(t + LAG + 16, 1000)
                t += 4

    nc.compile()
    return nc


def get_nc():
    if "nc" not in _CACHE:
        _CACHE["nc"] = _build()
    return _CACHE["nc"]


def prep_inputs(inputs):
    bf = ml_dtypes.bfloat16
    hs = np.asarray(inputs["hidden_states"], dtype=np.float32)
    mask = np.asarray(inputs["attention_mask"], dtype=np.float32)
    Wq = np.asarray(inputs["Wq"], np.float32)
    Wk = np.asarray(inputs["Wk"], np.float32)
    Wv = np.asarray(inputs["Wv"], np.float32)
    idx = int(np.asarray(inputs["index"]))
    bqf = (np.asarray(inputs["bq"], np.float32)
           + np.asarray(inputs["q_emb"], np.float32)[idx])
    bkf = (np.asarray(inputs["bk"], np.float32)
           + np.asarray(inputs["k_emb"], np.float32)[idx])
    bvf = (np.asarray(inputs["bv"], np.float32)
           + np.asarray(inputs["v_emb"], np.float32)[idx])
    scale = np.float32(1.0 / np.sqrt(HD))

    _CACHE["bvf"] = bvf
    in_maps = []
    for core in range(8):
        b, hg = divmod(core, 2)
        J = slice(hg * JC, (hg + 1) * JC)
        wq_s = np.ascontiguousarray(Wq[:, J] * scale).astype(bf)
        wk_s = np.ascontiguousarray(Wk[:, J]).astype(bf)
        wv_s = np.ascontiguousarray(Wv[:, J]).astype(bf)
        hsTb = np.ascontiguousarray(hs[b].T).astype(bf)
        emx = np.ascontiguousarray(
            np.exp(mask[b, 0, 0, :]).astype(np.float32).reshape(NTB, P).T)
        bq_sc = (bqf[J] * scale).astype(np.float32).reshape(4, P).T
        bk_c = bkf[J].astype(np.float32).reshape(4, P).T
        bqkc = np.ascontiguousarray(np.concatenate([bq_sc, bk_c], axis=1))
        in_maps.append({"hsT": hsTb, "wq": wq_s, "wk": wk_s,
                        "wv": wv_s, "em": emx, "bqk": bqkc})
    return in_maps


def postprocess_core(raw, dens):
    """raw: [512, 2048] unnormalized ctx^T (8 heads x 64 rows);
    dens: [32, 512] denominator, row = head*4 + qc.
    Returns [S, 512] normalized output columns for this core."""
    U = np.asarray(raw, np.float32).reshape(HPC, HD, S)
    denom = np.asarray(dens, np.float32).reshape(HPC, NQC * 512)
    ctxs = U / denom[:, None, :]
    return ctxs.transpose(2, 0, 1).reshape(S, HPC * HD)


def postprocess(results):
    bvf = _CACHE["bvf"]
    final = np.empty((B, S, H), np.float32)
    for core in range(8):
        b, hg = divmod(core, 2)
        J = slice(hg * JC, (hg + 1) * JC)
        final[b, :, J] = postprocess_core(
            results[core]["out"], results[core]["den"]) + bvf[None, J]
    return final


def _fast_run(nc, in_maps):
    """Repeat-call path: reuse one jitted SPMD executable."""
    import jax
    import concourse.mybir as mybir
    from concourse import bass2jax
    if "runner" not in _CACHE:
        from jax.experimental.shard_map import shard_map
        from jax.sharding import Mesh, PartitionSpec
        bass2jax.install_neuronx_cc_hook()
        pn = nc.partition_id_tensor.name if nc.partition_id_tensor else None
        in_names, out_names, out_avals, zero_outs = [], [], [], []
        for alloc in nc.m.functions[0].allocations:
            if not isinstance(alloc, mybir.MemoryLocationSet):
                continue
            name = alloc.memorylocations[0].name
            if alloc.kind == "ExternalInput":
                if name != pn:
                    in_names.append(name)
            elif alloc.kind == "ExternalOutput":
                out_names.append(name)
                shape = tuple(alloc.tensor_shape)
                dtype = mybir.dt.np(alloc.dtype)
                out_avals.append(jax.core.ShapedArray(shape, dtype))
                zero_outs.append(np.zeros(shape, dtype))
        alln = in_names + out_names + ([pn] if pn else [])

        def _body(*args):
            ops = list(args)
            if pn:
                ops.append(bass2jax.partition_id_tensor())
            return tuple(bass2jax._bass_exec_p.bind(
                *ops, out_avals=tuple(out_avals), in_names=tuple(alln),
                out_names=tuple(out_names), lowering_input_output_aliases=(),
                sim_require_finite=True, sim_require_nnan=True, nc=nc))

        mesh = Mesh(np.array(jax.devices()[:8]), ("core",))
        npar, nout = len(in_names), len(out_names)
        sharded = jax.jit(
            shard_map(_body, mesh=mesh,
                      in_specs=(PartitionSpec("core"),) * (npar + nout),
                      out_specs=(PartitionSpec("core"),) * nout,
                      check_rep=False),
            donate_argnums=tuple(range(npar, npar + nout)), keep_unused=True)
        _CACHE["runner"] = (sharded, in_names, out_names, out_avals, zero_outs)
    sharded, in_names, out_names, out_avals, zero_outs = _CACHE["runner"]
    cin = [np.concatenate([np.asarray(in_maps[c][nm]) for c in range(8)], 0)
           for nm in in_names]
    zs = [np.zeros((8 * z.shape[0], *z.shape[1:]), z.dtype)
          for z in zero_outs]
    outs = sharded(*cin, *zs)
    jax.block_until_ready(outs)
    return [{nm: np.asarray(outs[i]).reshape(8, *out_avals[i].shape)[c]
             for i, nm in enumerate(out_names)} for c in range(8)]


def kernel(**inputs):
    from concourse import bass_utils
    nc = get_nc()
    in_maps = prep_inputs(inputs)
    if _CACHE.get("ran_once"):
        results = _fast_run(nc, in_maps)
        return postprocess(results)
    res = bass_utils.run_bass_kernel_spmd(
        nc, in_maps, core_ids=list(range(8)),
        trace=_CACHE.get("trace", False))
    _CACHE["last_result"] = res
    _CACHE["ran_once"] = True
    return postprocess(res.results)
